# revision 8
# baseline (speedup 1.0000x reference)
"""Trainium2 Bass kernel for the blended-MoE actor network.

Math: reference computes, per sample,
    g1 = relu(bw1 @ s + bb1); g2 = relu(bw2 @ g1 + bb2)
    c  = softmax(bwo @ g2 + bbo)            # 2 experts
    h  = relu(blend(W1_e, s)); ...; mu = tanh(blend(Wm_e, h2))
with blend(W_e, x) = sum_e c_e (W_e x + b_e).

Since NE=2 and c0+c1=1:  c0 = sigmoid((bwo[0]-bwo[1]) @ g2 + dbo)  and
    blend(W_e, x) = W_1 x + b_1 + c0 * (dW x + db),  dW = W_0-W_1.
The c0 * (dW x) term is computed by scaling the matmul INPUT per-sample
(x_c = C0 .* x) so both expert contributions accumulate into one PSUM
group. Rank-1 bias terms c0*db are folded in as K=1 matmuls against the
broadcast C0 tile (or via an appended ones-row on the states).

Layout: activations are [features, batch] on-chip (host pre-transposes
states and appends a ones row). Batch is tiled at N=512 (one PSUM bank).
The router logit-diff matmul uses wd replicated across 128 output
columns so its PSUM output already holds the per-sample coefficient in
every partition row: one sigmoid yields the broadcast C0 tile for free.

Sharding: pure data parallel over 8 cores (batch 65536 -> 8 x 8192).
"""

import numpy as np

import concourse.bass as bass
import concourse.mybir as mybir
import concourse.tile as tile
from concourse import bacc
from concourse.bass_utils import run_bass_kernel_spmd

N_CORES = 8
B = 65536
BS = B // N_CORES  # 8192 per core
NI = 376  # state features
NIA = NI + 1  # + ones row
NA = 17  # actions
L1 = 256
L2 = 128
BH = 128  # blending hidden
NT = 512  # batch tile (matmul free dim)
T = BS // NT  # 16 tiles per core

F32 = mybir.dt.float32
# Storage dtype for activations/weights in SBUF and DRAM I/O of states.
DT = mybir.dt.float32r
DT_NP = np.float32
# Matmul compute dtype: float32r streams 1 row/cycle (vs 4 for float32).
MM_DT = mybir.dt.float32r

AF = mybir.ActivationFunctionType


def _mm(ap):
    return ap.bitcast(MM_DT) if MM_DT != DT else ap


# ---------------------------------------------------------------- weights
# All stationary operands are packed into one [128, WCOLS] host array;
# each lhsT is a column slice [0:K, off:off+M]. Rows >= K are zero.


class _Pack:
    def __init__(self):
        self.cols = []
        self.off = 0

    def add(self, arr):  # arr [K, M] -> returns (off, K, M)
        k, m = arr.shape
        assert k <= 128
        a = np.zeros((128, m), np.float32)
        a[:k] = arr
        off = self.off
        self.cols.append(a)
        self.off += m
        return (off, k, m)

    def data(self):
        return np.concatenate(self.cols, axis=1)


def _prep_weights(p, bw1, bb1, bw2, bb2, bwo, bbo, ew1, eb1, ew2, eb2, ewm, ebm):
    """Returns dict of packed-slice descriptors."""
    d = {}
    # blend L1: lhsT [377, 128] (= [bw1.T; bb1]) in 3 K-chunks
    w1a = np.concatenate([bw1.T, bb1[None, :]], axis=0)  # [377, 128]
    d["bl1"] = [p.add(w1a[k0:k1]) for k0, k1 in ((0, 128), (128, 256), (256, NIA))]
    # blend L2
    d["bl2"] = [p.add(bw2.T)]  # [128, 128]
    d["bb2"] = p.add(bb2[:, None])  # [128, 1] bias
    # router logit diff, replicated to 128 output columns
    wd = (bwo[0] - bwo[1])[:, None]  # [128, 1]
    d["wd"] = [p.add(np.repeat(wd, 128, axis=1))]  # [128, 128]
    d["bd"] = p.add(np.full((128, 1), bbo[0] - bbo[1], np.float32))
    # expert L1: base = expert1, diff = expert0 - expert1; aug with bias row
    e1b = np.concatenate([ew1[1].T, eb1[1][None, :]], axis=0)  # [377, 256]
    e1d = np.concatenate([(ew1[0] - ew1[1]).T, (eb1[0] - eb1[1])[None, :]], axis=0)
    ks = ((0, 128), (128, 256), (256, NIA))
    d["e1b"] = [[p.add(e1b[k0:k1, m : m + 128]) for k0, k1 in ks] for m in (0, 128)]
    d["e1d"] = [[p.add(e1d[k0:k1, m : m + 128]) for k0, k1 in ks] for m in (0, 128)]
    # expert L2
    e2b = ew2[1].T  # [256, 128]
    e2d = (ew2[0] - ew2[1]).T
    d["e2b"] = [p.add(e2b[0:128]), p.add(e2b[128:256])]
    d["e2d"] = [p.add(e2d[0:128]), p.add(e2d[128:256])]
    d["db2"] = [p.add((eb2[0] - eb2[1])[None, :])]  # [1, 128]
    d["b2"] = p.add(eb2[1][:, None])  # [128, 1]
    # expert out
    d["emb"] = [p.add(ewm[1].T)]  # [128, 17]
    d["emd"] = [p.add((ewm[0] - ewm[1]).T)]
    d["dbm"] = [p.add((ebm[0] - ebm[1])[None, :])]  # [1, 17]
    d["bm"] = p.add(ebm[1][:, None])  # [17, 1]
    return d


# ---------------------------------------------------------------- kernel


def _build(wd, wcols):
    """Build the Bass graph. wd: weight descriptors, wcols: pack width."""
    nc = bacc.Bacc("TRN2", target_bir_lowering=False, debug=False,
                   num_devices=N_CORES)
    xs = nc.declare_dram_parameter("xs", [NIA, BS], DT, isOutput=False)
    wk = nc.declare_dram_parameter("wk", [128, wcols], DT, isOutput=False)
    out = nc.declare_dram_parameter("out", [NA, BS], F32, isOutput=True)

    kchunks = ((0, 128), (128, 256), (256, NIA))

    with tile.TileContext(nc) as tc:
        with (
            tc.tile_pool(name="wpool", bufs=1) as wpool,
            tc.tile_pool(name="spool", bufs=4) as spool,
            tc.tile_pool(name="scpool", bufs=2) as scpool,
            tc.tile_pool(name="apool", bufs=2) as apool,
            tc.tile_pool(name="opool", bufs=2) as opool,
            tc.tile_pool(name="psum", bufs=1, space="PSUM") as pp,
        ):
            wkt = wpool.tile([128, wcols], DT)
            nc.sync.dma_start(wkt[:], wk[:])

            def W(desc):
                off, k, m = desc
                return wkt[0:k, off : off + m]

            for j in range(T):
                cs = slice(j * NT, (j + 1) * NT)
                # ---- load state chunks [128,NT] x2 + [121,NT]
                s = []
                for ci, (k0, k1) in enumerate(kchunks):
                    st = spool.tile([k1 - k0, NT], DT, tag=f"s{ci}")
                    nc.sync.dma_start(st[:], xs[k0:k1, cs])
                    s.append(st)

                # ---- blending MLP
                pg1 = pp.tile([BH, NT], F32, tag="g1")
                for ci in range(3):
                    nc.tensor.matmul(
                        pg1[:], _mm(W(wd["bl1"][ci])), _mm(s[ci][:]),
                        start=(ci == 0), stop=(ci == 2),
                    )
                g1 = apool.tile([BH, NT], DT, tag="g1")
                nc.scalar.activation(g1[:], pg1[:], AF.Relu)

                pg2 = pp.tile([BH, NT], F32, tag="g2")
                nc.tensor.matmul(pg2[:], _mm(W(wd["bl2"][0])), _mm(g1[:]),
                                 start=True, stop=True)
                g2 = apool.tile([BH, NT], DT, tag="g2")
                nc.scalar.activation(g2[:], pg2[:], AF.Relu, bias=W(wd["bb2"]))

                # router coeff, broadcast to all 128 rows by the wd-replicated
                # stationary: every row of pd equals the logit diff.
                pd = pp.tile([128, NT], F32, tag="d")
                nc.tensor.matmul(pd[:], _mm(W(wd["wd"][0])), _mm(g2[:]),
                                 start=True, stop=True)
                c0 = apool.tile([128, NT], DT, tag="c0")
                nc.scalar.activation(c0[:], pd[:], AF.Sigmoid, bias=W(wd["bd"]))

                # ---- scaled states (row 376 of xs is ones -> row 120 of
                # chunk 2 becomes c0 itself, feeding the diff bias column)
                sc = []
                for ci, (k0, k1) in enumerate(kchunks):
                    t = scpool.tile([k1 - k0, NT], DT, tag=f"sc{ci}")
                    nc.vector.tensor_mul(t[:], s[ci][:], c0[0 : k1 - k0, :])
                    sc.append(t)

                # ---- expert L1 (two output halves)
                h1 = []
                for m in range(2):
                    ph = pp.tile([128, NT], F32, tag=f"h1{m}")
                    for ci in range(3):
                        nc.tensor.matmul(ph[:], _mm(W(wd["e1b"][m][ci])),
                                         _mm(s[ci][:]),
                                         start=(ci == 0), stop=False)
                    for ci in range(3):
                        nc.tensor.matmul(ph[:], _mm(W(wd["e1d"][m][ci])),
                                         _mm(sc[ci][:]),
                                         start=False, stop=(ci == 2))
                    ht = apool.tile([128, NT], DT, tag=f"h1{m}")
                    nc.scalar.activation(ht[:], ph[:], AF.Relu)
                    h1.append(ht)

                # ---- expert L2
                h1c = []
                for m in range(2):
                    t = scpool.tile([128, NT], DT, tag=f"h1c{m}")
                    nc.vector.tensor_mul(t[:], h1[m][:], c0[:])
                    h1c.append(t)
                ph2 = pp.tile([128, NT], F32, tag="h2")
                nc.tensor.matmul(ph2[:], _mm(W(wd["e2b"][0])), _mm(h1[0][:]),
                                 start=True, stop=False)
                nc.tensor.matmul(ph2[:], _mm(W(wd["e2b"][1])), _mm(h1[1][:]),
                                 start=False, stop=False)
                nc.tensor.matmul(ph2[:], _mm(W(wd["e2d"][0])), _mm(h1c[0][:]),
                                 start=False, stop=False)
                nc.tensor.matmul(ph2[:], _mm(W(wd["e2d"][1])), _mm(h1c[1][:]),
                                 start=False, stop=False)
                # + c0 * db2 (rank-1, K=1 against the c0 row)
                nc.tensor.matmul(ph2[:], _mm(W(wd["db2"][0])), _mm(c0[0:1, :]),
                                 start=False, stop=True)
                h2 = apool.tile([128, NT], DT, tag="h2")
                nc.scalar.activation(h2[:], ph2[:], AF.Relu, bias=W(wd["b2"]))

                # ---- expert out
                h2c = scpool.tile([128, NT], DT, tag="h2c")
                nc.vector.tensor_mul(h2c[:], h2[:], c0[:])
                pmu = pp.tile([NA, NT], F32, tag="mu")
                nc.tensor.matmul(pmu[:], _mm(W(wd["emb"][0])), _mm(h2[:]),
                                 start=True, stop=False)
                nc.tensor.matmul(pmu[:], _mm(W(wd["emd"][0])), _mm(h2c[:]),
                                 start=False, stop=False)
                nc.tensor.matmul(pmu[:], _mm(W(wd["dbm"][0])), _mm(c0[0:1, :]),
                                 start=False, stop=True)
                mu = opool.tile([NA, NT], F32, tag="mu")
                nc.scalar.activation(mu[:], pmu[:], AF.Tanh, bias=W(wd["bm"]))
                nc.sync.dma_start(out[:, cs], mu[:])
    nc.finalize()
    return nc


_CACHE = {}


def kernel(**inputs) -> np.ndarray:
    states = np.asarray(inputs["states"], np.float32)
    pack = _Pack()
    wdesc = _prep_weights(
        pack,
        *[
            np.asarray(inputs[k], np.float32)
            for k in ("bw1", "bb1", "bw2", "bb2", "bwo", "bbo",
                      "ew1", "eb1", "ew2", "eb2", "ewm", "ebm")
        ],
    )
    wdata = pack.data().astype(DT_NP)  # [128, wcols]

    if "nc" not in _CACHE:
        _CACHE["nc"] = _build(wdesc, wdata.shape[1])
    nc = _CACHE["nc"]

    in_maps = []
    for c in range(N_CORES):
        shard = states[c * BS : (c + 1) * BS]  # [BS, NI]
        xs = np.empty((NIA, BS), np.float32)
        xs[:NI] = shard.T
        xs[NI] = 1.0
        in_maps.append({"xs": xs.astype(DT_NP), "wk": wdata})

    res = run_bass_kernel_spmd(nc, in_maps, core_ids=list(range(N_CORES)))
    out = np.empty((B, NA), np.float32)
    for c in range(N_CORES):
        out[c * BS : (c + 1) * BS] = res.results[c]["out"].T
    return out
